# revision 1
# baseline (speedup 1.0000x reference)
"""CommNet forward kernel for 8 Trainium2 NeuronCores.

Reference computation (per sample of N=32 agents, batch B=16384):
    h   = relu(obs @ enc_w + enc_b)                    # [B,N,64]
    2x:  msg = (sum_n h - h)/31
         h   = relu(concat(h, msg) @ comm_w[r] + comm_b[r])
    hid = relu(h @ out_w1 + out_b1)
    q   = hid @ out_w2 + out_b2; q[avail==0] = -1e10

Device strategy (pure data parallel, batch split 8 ways):
  * activations feature-major [feat(part), row(free)]; four 512-row
    groups per 2048-row super-iteration, packed 2x2 into the PE array
    via tile_position (K=64, M=64 quadrants) so all 16 subarrays
    compute concurrently.  Groups at (p-half, f-half) positions
    (0,1)/(1,0) swap every matmul layer; 4 permuting layers = identity,
    so the out2 layout matches the obs layout.
  * comm round rewritten as h @ W_self + S @ W_sum with
    W_self = W_h - W_m/31, W_sum = W_m/31, S = per-sample agent sum.
    S comes from identity-weight matmuls with a step-0 (broadcast)
    output AP that accumulates the 32 agent columns of each sample into
    one PSUM column; the S @ W_sum term re-broadcasts S via a step-0
    rhs AP into the same accumulation group as the W_self matmul.
  * relu+bias fused into the PSUM->SBUF evacuation (DVE dual-op
    tensor_scalar for enc/out1, ScalarE activation for the rounds)
  * mask+final bias folded host-side into pen = where(avail, out_b2, -1e10);
    pen is added on the PE (identity-lhsT matmul accumulate) and the q
    bank evacuated with a ScalarE copy
  * host pre-packs obs into the feature-major layout and unpacks q
    (layout work is free on host; the device does all the FLOPs)
"""

import contextlib
import sys

import numpy as np

sys.path.insert(0, "/opt/trn_rl_repo")

import ml_dtypes  # noqa: E402

B, N, OBS, H, A, NR = 16384, 32, 64, 64, 16, 2
NCORES = 8
RPC = B * N // NCORES   # rows per core = 65536

SUP = 2048              # rows per super-iteration (4 groups of 512)
GRP = 512               # rows per group (one fp32 PSUM bank)
NSUP = RPC // SUP
NS_G = GRP // N         # samples per group = 16
NS_H = 2 * NS_G         # samples per partition-half per super = 32

_cache = {}


def _build_device_program():
    import concourse.bacc as bacc
    import concourse.mybir as mybir
    from concourse import tile

    F32 = mybir.dt.float32
    BF16 = mybir.dt.bfloat16

    nc = bacc.Bacc("TRN2", target_bir_lowering=False, debug=False)

    obs_d = nc.dram_tensor("obs_pk", [NSUP, 128, SUP // 2], BF16, kind="ExternalInput")
    pen_d = nc.dram_tensor("pen_pk", [NSUP // 2, 128, GRP], F32, kind="ExternalInput")
    q_d = nc.dram_tensor("q_pk", [NSUP // 2, 128, GRP], BF16, kind="ExternalOutput")

    # replicated-on-both-halves [128, 64] weights; W2 block-diag [128, 32]
    wname = ["Wenc", "Wself0", "Wself1", "Wsum0", "Wsum1", "W1", "idn"]
    w_d = {n: nc.dram_tensor(n, [128, 64], BF16, kind="ExternalInput") for n in wname}
    w_d["W2"] = nc.dram_tensor("W2", [128, 32], BF16, kind="ExternalInput")
    w_d["idnq"] = nc.dram_tensor("idnq", [128, 32], F32, kind="ExternalInput")
    bname = ["be", "b0", "b1", "bh"]
    b_d = {n: nc.dram_tensor(n, [128, 1], F32, kind="ExternalInput") for n in bname}

    FD = GRP
    Relu = mybir.ActivationFunctionType.Relu
    Copy = mybir.ActivationFunctionType.Copy
    ALU = mybir.AluOpType
    QUAD = [(0, 0, 0, 0), (0, 1, 0, 64), (1, 0, 64, 64), (1, 1, 64, 0)]
    # (in p-half, in f-half, rhs part base, out part base); out f-half = in f-half
    # after act: group at (ph, fh) lands at (out_base//64, fh) -> (0,1)/(1,0) swap

    with tile.TileContext(nc) as tc, contextlib.ExitStack() as ctx:
        wp = ctx.enter_context(tc.tile_pool(name="w", bufs=1))
        pool = ctx.enter_context(tc.tile_pool(name="p", bufs=3))
        psum = ctx.enter_context(tc.tile_pool(name="ps", bufs=1, space="PSUM"))

        W = {}
        for n in wname:
            W[n] = wp.tile([128, 64], BF16, tag=n, name=f"w_{n}")
            nc.sync.dma_start(W[n][:], w_d[n][:])
        W["W2"] = wp.tile([128, 32], BF16, tag="W2", name="w_W2")
        nc.sync.dma_start(W["W2"][:], w_d["W2"][:])
        W["idnq"] = wp.tile([128, 32], F32, tag="idnq", name="w_idnq")
        nc.sync.dma_start(W["idnq"][:], w_d["idnq"][:])
        BIAS = {}
        for n in bname:
            BIAS[n] = wp.tile([128, 1], F32, tag=n, name=f"b_{n}")
            nc.sync.dma_start(BIAS[n][:], b_d[n][:])

        def layer_mms(ps, wt, rhs_t):
            """4 concurrent K=64/M=64 matmuls (one per group) into ps[128,1024]."""
            for ph, fh, rb, ob in QUAD:
                nc.tensor.matmul(
                    ps[ob:ob + 64, fh * FD:(fh + 1) * FD],
                    wt[rb:rb + 64, :],
                    rhs_t[ph * 64:(ph + 1) * 64, fh * FD:(fh + 1) * FD],
                    start=True, stop=True, tile_position=(rb, ob),
                )

        for s in range(NSUP):
            obs_t = pool.tile([128, 2 * FD], BF16, tag="obs")
            nc.sync.dma_start(obs_t[:], obs_d[s])

            psE = psum.tile([128, 2 * FD], F32, tag="stg", bufs=3)
            layer_mms(psE, W["Wenc"], obs_t)
            h = pool.tile([128, 2 * FD], BF16, tag="h0")
            nc.vector.tensor_scalar(h[:], psE[:], BIAS["be"][:], 0.0,
                                    ALU.add, ALU.max)

            for r in range(NR):
                psS = psum.tile([128, NS_H], F32, tag="S")
                for hp, tp in ((0, 0), (64, 64)):
                    for sh in range(2):
                        rhs = h[hp:hp + 64, sh * FD:(sh + 1) * FD] \
                            .rearrange("p (S n) -> p n S", n=N)
                        outS = psS[hp:hp + 64, sh * NS_G:(sh + 1) * NS_G] \
                            .unsqueeze(1).broadcast_to([64, N, NS_G])
                        nc.tensor.matmul(outS, W["idn"][hp:hp + 64, :], rhs,
                                         start=True, stop=True,
                                         tile_position=(tp, tp))
                S2 = pool.tile([128, NS_H], BF16, tag="S2")
                nc.vector.tensor_copy(S2[:], psS[:])

                psR = psum.tile([128, 2 * FD], F32, tag="stg", bufs=3)
                for ph, fh, rb, ob in QUAD:
                    nc.tensor.matmul(
                        psR[ob:ob + 64, fh * FD:(fh + 1) * FD],
                        W[f"Wself{r}"][rb:rb + 64, :],
                        h[ph * 64:(ph + 1) * 64, fh * FD:(fh + 1) * FD],
                        start=True, stop=False, tile_position=(rb, ob),
                    )
                    sb = S2[ph * 64:(ph + 1) * 64, fh * NS_G:(fh + 1) * NS_G] \
                        .unsqueeze(2).broadcast_to([64, NS_G, N])
                    nc.tensor.matmul(
                        psR[ob:ob + 64, fh * FD:(fh + 1) * FD],
                        W[f"Wsum{r}"][rb:rb + 64, :], sb,
                        start=False, stop=True, tile_position=(rb, ob),
                    )
                h = pool.tile([128, 2 * FD], BF16, tag=f"h{1 + r}")
                nc.scalar.activation(h[:], psR[:], Relu, bias=BIAS[f"b{r}"][:])

            psH = psum.tile([128, 2 * FD], F32, tag="stg", bufs=3)
            layer_mms(psH, W["W1"], h)
            hid = pool.tile([128, 2 * FD], BF16, tag="hid")
            nc.vector.tensor_scalar(hid[:], psH[:], BIAS["bh"][:], 0.0,
                                    ALU.add, ALU.max)

            # out2: block-diag over partition pairs; two col positions.
            # q banks of even/odd super-iters pack into one [128, FD] bank
            # (partition halves) so the evacuation runs full-width half as often.
            k = s % 2
            qo = 64 * k
            if k == 0:
                pen_t = pool.tile([128, FD], F32, tag="pen")
                nc.sync.dma_start(pen_t[:], pen_d[s // 2])
                psQ = psum.tile([128, FD], F32, tag="q")
                pers = (pen_t, psQ)
            else:
                pen_t, psQ = pers
            nc.tensor.matmul(psQ[qo:qo + 32, :], W["W2"][:], hid[:, 0:FD],
                             start=True, stop=False, tile_position=(0, qo),
                             skip_group_check=True)
            nc.tensor.matmul(psQ[qo:qo + 32, :], W["idnq"][qo:qo + 32, :],
                             pen_t[qo:qo + 32, :],
                             start=False, stop=True, tile_position=(qo % 128 // 32 * 32, qo),
                             skip_group_check=True)
            nc.tensor.matmul(psQ[qo + 32:qo + 64, :], W["W2"][:], hid[:, FD:2 * FD],
                             start=True, stop=False, tile_position=(0, qo + 32),
                             skip_group_check=True)
            nc.tensor.matmul(psQ[qo + 32:qo + 64, :], W["idnq"][qo + 32:qo + 64, :],
                             pen_t[qo + 32:qo + 64, :],
                             start=False, stop=True,
                             tile_position=((qo + 32) % 128 // 32 * 32, qo + 32),
                             skip_group_check=True)
            if k == 1:
                q_sb = pool.tile([128, FD], BF16, tag="qsb")
                nc.scalar.activation(q_sb[:], psQ[:], Copy)
                nc.sync.dma_start(q_d[s // 2], q_sb[:])

    nc.compile()
    return nc


def _prep_host(obs, enc_w, enc_b, comm_w, comm_b, out_w1, out_b1, out_w2, out_b2,
               available_actions):
    """Build per-core input maps (packed layouts + derived weights)."""
    bf16 = ml_dtypes.bfloat16
    f32 = np.float32

    def rep(w):  # replicate [64, m] weight onto both partition halves
        return np.ascontiguousarray(np.concatenate([w, w], axis=0)
                                    .astype(f32)).astype(bf16)

    def bd(w):  # block-diag duplicate [k,m] -> [2k, 2m]
        k, m = w.shape
        o = np.zeros((2 * k, 2 * m), f32)
        o[:k, :m] = w
        o[k:, m:] = w
        return np.ascontiguousarray(o).astype(bf16)

    weights = {"Wenc": rep(enc_w), "W1": rep(out_w1), "W2": bd(out_w2),
               "idn": rep(np.eye(64, dtype=f32)),
               "idnq": np.ascontiguousarray(np.tile(np.eye(32, dtype=f32), (4, 1)))}
    for r in range(NR):
        wh = comm_w[r][:H].astype(f32)
        wm = comm_w[r][H:].astype(f32) / (N - 1)
        weights[f"Wself{r}"] = rep(wh - wm)
        weights[f"Wsum{r}"] = rep(wm)
    biases = {"be": enc_b, "b0": comm_b[0], "b1": comm_b[1], "bh": out_b1}
    biases = {k: np.concatenate([v, v]).astype(f32).reshape(128, 1)
              for k, v in biases.items()}

    rows = np.ascontiguousarray(obs.reshape(B * N, OBS))
    pen = np.where(available_actions.reshape(B * N, A) == 0,
                   f32(-1e10), out_b2.astype(f32)[None, :]).astype(f32)

    in_maps = []
    for c in range(NCORES):
        ro = rows[c * RPC:(c + 1) * RPC]
        # [NSUP, phalf, fhalf, row, feat] -> [NSUP, phalf*feat, fhalf*row]
        opk = ro.reshape(NSUP, 2, 2, GRP, OBS).transpose(0, 1, 4, 2, 3) \
                .reshape(NSUP, 128, SUP // 2).astype(bf16)
        pe = pen[c * RPC:(c + 1) * RPC]
        # q/pen partitions: [fhalf, phalf, action]
        ppk = pe.reshape(NSUP, 2, 2, GRP, A).transpose(0, 2, 1, 4, 3) \
                .reshape(NSUP // 2, 128, GRP).astype(f32)
        m = {"obs_pk": np.ascontiguousarray(opk),
             "pen_pk": np.ascontiguousarray(ppk)}
        m.update(weights)
        m.update(biases)
        in_maps.append(m)
    return in_maps


def _unpack_output(results):
    qs = []
    for r in results:
        qpk = np.asarray(r["q_pk"]).astype(np.float32)  # [NSUP//2, 128, GRP]
        q = qpk.reshape(NSUP, 2, 2, A, GRP).transpose(0, 2, 1, 4, 3) \
               .reshape(RPC, A)
        qs.append(q)
    return np.concatenate(qs, axis=0).reshape(B, N, A)


def run_on_device(in_maps, trace=False):
    from concourse.bass_utils import run_bass_kernel_spmd

    if "nc" not in _cache:
        _cache["nc"] = _build_device_program()
    return run_bass_kernel_spmd(_cache["nc"], in_maps,
                                core_ids=list(range(NCORES)), trace=trace)


def kernel(obs, enc_w, enc_b, comm_w, comm_b, out_w1, out_b1, out_w2, out_b2,
           available_actions):
    args = [np.asarray(x) for x in
            (obs, enc_w, enc_b, comm_w, comm_b, out_w1, out_b1, out_w2, out_b2,
             available_actions)]
    in_maps = _prep_host(*args)
    res = run_on_device(in_maps)
    return _unpack_output(res.results)



# revision 11
# speedup vs baseline: 1.1073x; 1.1073x over previous
"""CommNet forward kernel for 8 Trainium2 NeuronCores.

Reference computation (per sample of N=32 agents, batch B=16384):
    h   = relu(obs @ enc_w + enc_b)                    # [B,N,64]
    2x:  msg = (sum_n h - h)/31
         h   = relu(concat(h, msg) @ comm_w[r] + comm_b[r])
    hid = relu(h @ out_w1 + out_b1)
    q   = hid @ out_w2 + out_b2; q[avail==0] = -1e10

Device strategy (pure data parallel, batch split 8 ways):
  * activations feature-major [feat(part), row(free)]; four 512-row
    groups per 2048-row super-iteration, packed 2x2 into the PE array
    via tile_position (K=64, M=64 quadrants) so all 16 subarrays
    compute concurrently.  Groups at (p-half, f-half) positions
    (0,1)/(1,0) swap every matmul layer; 4 permuting layers = identity,
    so the out2 layout matches the obs layout.
  * comm round rewritten as h @ W_self + S @ W_sum with
    W_self = W_h - W_m/31, W_sum = W_m/31, S = per-sample agent sum.
    S comes from identity-weight matmuls with a step-0 (broadcast)
    output AP that accumulates the 32 agent columns of each sample into
    one PSUM column; group (ph, fh) sums land on the fh partition-half
    at column ph*16 so the 4 S-matmuls cover all 4 quadrant positions.
    The S @ W_sum term re-broadcasts S via a step-0 rhs AP into the
    same accumulation group as the W_self matmul (also 4 positions).
  * relu+bias fused into the PSUM->SBUF evacuation (DVE dual-op
    tensor_scalar for enc/out1, ScalarE activation for the rounds;
    S copies split DVE/ScalarE to balance engine load)
  * mask+final bias folded host-side into pen = where(avail, out_b2, -1e10),
    stored bf16; pen is added on the PE with a single K=64 identity-lhsT
    bf16 matmul per super (bf16 streams 1 col/cyc vs 4 for f32) and the
    q bank evacuated with a ScalarE copy
  * host pre-packs obs into the feature-major layout and unpacks q
    (layout work is free on host; the device does all the FLOPs)
"""

import contextlib
import sys

import numpy as np

sys.path.insert(0, "/opt/trn_rl_repo")

import ml_dtypes  # noqa: E402

B, N, OBS, H, A, NR = 16384, 32, 64, 64, 16, 2
NCORES = 8
RPC = B * N // NCORES   # rows per core = 65536

SUP = 2048              # rows per super-iteration (4 groups of 512)
GRP = 512               # rows per group (one fp32 PSUM bank)
NSUP = RPC // SUP
NS_G = GRP // N         # samples per group = 16
NS_H = 2 * NS_G         # samples per partition-half per super = 32

_cache = {}


def _build_device_program():
    import concourse.bacc as bacc
    import concourse.mybir as mybir
    from concourse import tile

    F32 = mybir.dt.float32
    BF16 = mybir.dt.bfloat16

    nc = bacc.Bacc("TRN2", target_bir_lowering=False, debug=False)

    obs_d = nc.dram_tensor("obs_pk", [NSUP, 128, SUP // 2], BF16, kind="ExternalInput")
    pen_d = nc.dram_tensor("pen_pk", [NSUP // 2, 128, GRP], BF16, kind="ExternalInput")
    q_d = nc.dram_tensor("q_pk", [NSUP // 2, 128, GRP], BF16, kind="ExternalOutput")

    # replicated-on-both-halves [128, 64] weights; W2 block-diag [128, 32]
    wname = ["Wenc", "Wself0", "Wself1", "Wsum0", "Wsum1", "W1", "idn"]
    w_d = {n: nc.dram_tensor(n, [128, 64], BF16, kind="ExternalInput") for n in wname}
    w_d["W2"] = nc.dram_tensor("W2", [128, 32], BF16, kind="ExternalInput")
    w_d["idnq"] = nc.dram_tensor("idnq", [128, 64], BF16, kind="ExternalInput")
    bname = ["be", "b0", "b1", "bh"]
    b_d = {n: nc.dram_tensor(n, [128, 1], F32, kind="ExternalInput") for n in bname}

    FD = GRP
    Relu = mybir.ActivationFunctionType.Relu
    Copy = mybir.ActivationFunctionType.Copy
    ALU = mybir.AluOpType
    QUAD = [(0, 0, 0, 0), (0, 1, 0, 64), (1, 0, 64, 64), (1, 1, 64, 0)]
    # (in p-half, in f-half, rhs part base, out part base); out f-half = in f-half
    # after act: group at (ph, fh) lands at (out_base//64, fh) -> (0,1)/(1,0) swap

    with tile.TileContext(nc) as tc, contextlib.ExitStack() as ctx:
        wp = ctx.enter_context(tc.tile_pool(name="w", bufs=1))
        pool = ctx.enter_context(tc.tile_pool(name="p", bufs=3))
        psum = ctx.enter_context(tc.tile_pool(name="ps", bufs=1, space="PSUM"))

        W = {}
        for n in wname:
            W[n] = wp.tile([128, 64], BF16, tag=n, name=f"w_{n}")
            nc.sync.dma_start(W[n][:], w_d[n][:])
        W["W2"] = wp.tile([128, 32], BF16, tag="W2", name="w_W2")
        nc.sync.dma_start(W["W2"][:], w_d["W2"][:])
        W["idnq"] = wp.tile([128, 64], BF16, tag="idnq", name="w_idnq")
        nc.sync.dma_start(W["idnq"][:], w_d["idnq"][:])
        BIAS = {}
        for n in bname:
            BIAS[n] = wp.tile([128, 1], F32, tag=n, name=f"b_{n}")
            nc.sync.dma_start(BIAS[n][:], b_d[n][:])

        def layer_mms(ps, wt, rhs_t):
            """4 concurrent K=64/M=64 matmuls (one per group) into ps[128,1024]."""
            for ph, fh, rb, ob in QUAD:
                nc.tensor.matmul(
                    ps[ob:ob + 64, fh * FD:(fh + 1) * FD],
                    wt[rb:rb + 64, :],
                    rhs_t[ph * 64:(ph + 1) * 64, fh * FD:(fh + 1) * FD],
                    start=True, stop=True, tile_position=(rb, ob),
                )

        for s in range(NSUP):
            obs_t = pool.tile([128, 2 * FD], BF16, tag="obs")
            nc.sync.dma_start(obs_t[:], obs_d[s])

            psE = psum.tile([128, 2 * FD], F32, tag="stg", bufs=3)
            layer_mms(psE, W["Wenc"], obs_t)
            h = pool.tile([128, 2 * FD], BF16, tag="h0")
            nc.vector.tensor_scalar(h[:], psE[:], BIAS["be"][:], 0.0,
                                    ALU.add, ALU.max)

            for r in range(NR):
                # S (per-sample agent sums) via identity matmuls on the two
                # diagonal positions (each partition half gets exactly one
                # concurrent writer -- PE has one PSUM write port/partition)
                psS = psum.tile([128, NS_H], F32, tag="S")
                for hp, tp in ((0, 0), (64, 64)):
                    for sh in range(2):
                        rhs = h[hp:hp + 64, sh * FD:(sh + 1) * FD] \
                            .rearrange("p (S n) -> p n S", n=N)
                        outS = psS[hp:hp + 64, sh * NS_G:(sh + 1) * NS_G] \
                            .unsqueeze(1).broadcast_to([64, N, NS_G])
                        nc.tensor.matmul(outS, W["idn"][hp:hp + 64, :], rhs,
                                         start=True, stop=True,
                                         tile_position=(tp, tp))
                S2 = pool.tile([128, NS_H], BF16, tag="S2")
                if r == 0:
                    nc.vector.tensor_copy(S2[:], psS[:])
                else:
                    nc.scalar.activation(S2[:], psS[:], Copy)

                # all Wself matmuls first: they only need h, while Wsum waits
                # on the S2 copy — PE dispatch is FIFO, so a waiting Wsum
                # would stall later Wself matmuls if interleaved
                psR = psum.tile([128, 2 * FD], F32, tag="stg", bufs=3)
                for ph, fh, rb, ob in QUAD:
                    nc.tensor.matmul(
                        psR[ob:ob + 64, fh * FD:(fh + 1) * FD],
                        W[f"Wself{r}"][rb:rb + 64, :],
                        h[ph * 64:(ph + 1) * 64, fh * FD:(fh + 1) * FD],
                        start=True, stop=False, tile_position=(rb, ob),
                    )
                for ph, fh, rb, ob in QUAD:
                    sb = S2[ph * 64:(ph + 1) * 64, fh * NS_G:(fh + 1) * NS_G] \
                        .unsqueeze(2).broadcast_to([64, NS_G, N])
                    nc.tensor.matmul(
                        psR[ob:ob + 64, fh * FD:(fh + 1) * FD],
                        W[f"Wsum{r}"][rb:rb + 64, :], sb,
                        start=False, stop=True, tile_position=(rb, ob),
                    )
                h = pool.tile([128, 2 * FD], BF16, tag=f"h{1 + r}")
                nc.scalar.activation(h[:], psR[:], Relu, bias=BIAS[f"b{r}"][:])

            psH = psum.tile([128, 2 * FD], F32, tag="stg", bufs=3)
            layer_mms(psH, W["W1"], h)
            hid = pool.tile([128, 2 * FD], BF16, tag="hid")
            nc.vector.tensor_scalar(hid[:], psH[:], BIAS["bh"][:], 0.0,
                                    ALU.add, ALU.max)

            # out2: block-diag over partition pairs; two col positions.
            # q banks of even/odd super-iters pack into one [128, FD] bank
            # (partition halves) so the evacuation runs full-width half as often.
            k = s % 2
            qo = 64 * k
            if k == 0:
                pen_t = pool.tile([128, FD], BF16, tag="pen")
                nc.sync.dma_start(pen_t[:], pen_d[s // 2])
                psQ = psum.tile([128, FD], F32, tag="q")
                pers = (pen_t, psQ)
            else:
                pen_t, psQ = pers
            nc.tensor.matmul(psQ[qo:qo + 32, :], W["W2"][:], hid[:, 0:FD],
                             start=True, stop=False, tile_position=(0, qo),
                             skip_group_check=True)
            nc.tensor.matmul(psQ[qo + 32:qo + 64, :], W["W2"][:], hid[:, FD:2 * FD],
                             start=True, stop=False, tile_position=(0, qo + 32),
                             skip_group_check=True)
            # pen (mask/bias) add, one K=64 identity matmul covering both
            # 32-row W2 groups (bf16: 1 cyc/col vs 4 for the old f32 pair)
            nc.tensor.matmul(psQ[qo:qo + 64, :], W["idnq"][qo:qo + 64, :],
                             pen_t[qo:qo + 64, :],
                             start=False, stop=True, tile_position=(qo, qo),
                             skip_group_check=True)
            if k == 1:
                q_sb = pool.tile([128, FD], BF16, tag="qsb")
                nc.scalar.activation(q_sb[:], psQ[:], Copy)
                nc.sync.dma_start(q_d[s // 2], q_sb[:])

    nc.compile()
    return nc


def _prep_host(obs, enc_w, enc_b, comm_w, comm_b, out_w1, out_b1, out_w2, out_b2,
               available_actions):
    """Build per-core input maps (packed layouts + derived weights)."""
    bf16 = ml_dtypes.bfloat16
    f32 = np.float32

    def rep(w):  # replicate [64, m] weight onto both partition halves
        return np.ascontiguousarray(np.concatenate([w, w], axis=0)
                                    .astype(f32)).astype(bf16)

    def bd(w):  # block-diag duplicate [k,m] -> [2k, 2m]
        k, m = w.shape
        o = np.zeros((2 * k, 2 * m), f32)
        o[:k, :m] = w
        o[k:, m:] = w
        return np.ascontiguousarray(o).astype(bf16)

    weights = {"Wenc": rep(enc_w), "W1": rep(out_w1), "W2": bd(out_w2),
               "idn": rep(np.eye(64, dtype=f32)),
               "idnq": rep(np.eye(64, dtype=f32))}
    for r in range(NR):
        wh = comm_w[r][:H].astype(f32)
        wm = comm_w[r][H:].astype(f32) / (N - 1)
        weights[f"Wself{r}"] = rep(wh - wm)
        weights[f"Wsum{r}"] = rep(wm)
    biases = {"be": enc_b, "b0": comm_b[0], "b1": comm_b[1], "bh": out_b1}
    biases = {k: np.concatenate([v, v]).astype(f32).reshape(128, 1)
              for k, v in biases.items()}

    rows = np.ascontiguousarray(obs.reshape(B * N, OBS))
    pen = np.where(available_actions.reshape(B * N, A) == 0,
                   f32(-1e10), out_b2.astype(f32)[None, :]).astype(f32)
    pen = pen.astype(bf16)

    in_maps = []
    for c in range(NCORES):
        ro = rows[c * RPC:(c + 1) * RPC]
        # [NSUP, phalf, fhalf, row, feat] -> [NSUP, phalf*feat, fhalf*row]
        opk = ro.reshape(NSUP, 2, 2, GRP, OBS).transpose(0, 1, 4, 2, 3) \
                .reshape(NSUP, 128, SUP // 2).astype(bf16)
        pe = pen[c * RPC:(c + 1) * RPC]
        # q/pen partitions: [fhalf, phalf, action]
        ppk = pe.reshape(NSUP, 2, 2, GRP, A).transpose(0, 2, 1, 4, 3) \
                .reshape(NSUP // 2, 128, GRP)
        m = {"obs_pk": np.ascontiguousarray(opk),
             "pen_pk": np.ascontiguousarray(ppk)}
        m.update(weights)
        m.update(biases)
        in_maps.append(m)
    return in_maps


def _unpack_output(results):
    qs = []
    for r in results:
        qpk = np.asarray(r["q_pk"]).astype(np.float32)  # [NSUP//2, 128, GRP]
        q = qpk.reshape(NSUP, 2, 2, A, GRP).transpose(0, 2, 1, 4, 3) \
               .reshape(RPC, A)
        qs.append(q)
    return np.concatenate(qs, axis=0).reshape(B, N, A)


def run_on_device(in_maps, trace=False):
    from concourse.bass_utils import run_bass_kernel_spmd

    if "nc" not in _cache:
        _cache["nc"] = _build_device_program()
    return run_bass_kernel_spmd(_cache["nc"], in_maps,
                                core_ids=list(range(NCORES)), trace=trace)


def kernel(obs, enc_w, enc_b, comm_w, comm_b, out_w1, out_b1, out_w2, out_b2,
           available_actions):
    args = [np.asarray(x) for x in
            (obs, enc_w, enc_b, comm_w, comm_b, out_w1, out_b1, out_w2, out_b2,
             available_actions)]
    in_maps = _prep_host(*args)
    res = run_on_device(in_maps)
    return _unpack_output(res.results)

